# revision 1
# baseline (speedup 1.0000x reference)
"""XNOR-Net++ style binary double-conv forward for Trainium2, 8 NeuronCores.

Computes, for x:[32,256,56,56] f32, w1,w2:[256,256,3,3] f32:
    alpha = (mean|w1| + mean|w2|)/2 over (I,kh,kw)            -> [O]
    out   = (conv(sign(x), sign(w1)) + conv(sign(x), sign(w2))) * alpha

Key algebraic fold: conv(s, sign(w1)) + conv(s, sign(w2)) ==
conv(s, sign(w1)+sign(w2)); the combined weight is in {-2,0,2} and the
activations in {-1,0,1}, both exact in fp8e4, so the conv's integer
part is reproduced exactly and only the final alpha multiply rounds.

Implementation: 9 shifted-tap matmuls accumulate each output tile in
PSUM. fp8 + DoubleRow folds the K=256 contraction (2x128 C_in chunks)
into single matmuls. The padded sign image is stored flat ([58*58] per
C_in chunk, 58-wide rows, one guard byte on each end) so every tap
window is a contiguous 464-element slice; the wrap-around columns land
in the two padding columns of each 8-row output block and are dropped
by the PSUM->SBUF scale-copy.

Scheduling: weight loads go first (they gate the PE transposes that
build the 18 stationary DoubleRow tiles), x loads are software-
prefetched ahead on the gpsimd/sync/scalar queues, PSUM->SBUF scale-
copies alternate DVE/ScalarE, and output DMAs ride the sync queue.
Padding borders are zeroed with three small strided memsets per C_in
plane instead of a full-tile memset.

Sharding: data-parallel over batch, 4 images per core, weights
replicated; outputs concatenated on host.
"""

import numpy as np

P = 128
H = W = 56
WP = 58  # padded row width
PLANE = WP * WP  # 3364 flat padded plane
PLANE_STRIDE = 3376  # 16-aligned (DoubleRow AP step%16==0), >= 1+PLANE+1
NIMG = 4  # images per core
NCORES = 8
RB = 8  # output rows per matmul block
NBLK = H // RB  # 7
N_FREE = RB * WP  # 464 <= 512 (one PSUM bank)

_CACHE = {}


def _build_program():
    from contextlib import ExitStack

    import concourse.bacc as bacc
    import concourse.mybir as mybir
    import concourse.tile as tile
    from concourse.masks import make_identity

    dt = mybir.dt
    AF = mybir.ActivationFunctionType

    nc = bacc.Bacc(
        "TRN2",
        target_bir_lowering=False,
        debug=False,
        num_devices=NCORES,
    )
    x = nc.dram_tensor("x", [NIMG, 256, H, W], dt.float32, kind="ExternalInput").ap()
    w1 = nc.dram_tensor("w1", [256, 256, 3, 3], dt.float32, kind="ExternalInput").ap()
    w2 = nc.dram_tensor("w2", [256, 256, 3, 3], dt.float32, kind="ExternalInput").ap()
    out = nc.dram_tensor(
        "out", [NIMG, 256, H, W], dt.float32, kind="ExternalOutput"
    ).ap()

    with tile.TileContext(nc) as tc, ExitStack() as ctx:
        consts = ctx.enter_context(tc.tile_pool(name="consts", bufs=1))
        wprep = ctx.enter_context(tc.tile_pool(name="wprep", bufs=2))
        xraw_pool = ctx.enter_context(tc.tile_pool(name="xraw", bufs=4))
        xpad_pool = ctx.enter_context(tc.tile_pool(name="xpad", bufs=3))
        psum_pool = ctx.enter_context(tc.tile_pool(name="psum", bufs=7, space="PSUM"))
        outp = ctx.enter_context(tc.tile_pool(name="outp", bufs=4))

        ident = consts.tile([P, P], dt.bfloat16)
        make_identity(nc, ident)

        # alpha per output channel, one column per oc chunk
        alpha_sb = consts.tile([P, 2], dt.float32)
        # per-(oc,tap) stationary weight tiles [i_local, ic, o_local], fp8
        lhsT_t = [
            [
                consts.tile([P, 2, P], dt.float8e4, name=f"lhsT_{oc}_{tap}")
                for tap in range(9)
            ]
            for oc in range(2)
        ]

        # ---- weight DMAs first: small, on the critical path ----
        wr = [[None, None], [None, None]]
        for oc in range(2):
            wr1 = wprep.tile([P, 256, 3, 3], dt.float32, tag="wraw", name=f"wr1_{oc}")
            wr2 = wprep.tile([P, 256, 3, 3], dt.float32, tag="wraw2", name=f"wr2_{oc}")
            nc.sync.dma_start(out=wr1, in_=w1[oc * P : (oc + 1) * P])
            nc.scalar.dma_start(out=wr2, in_=w2[oc * P : (oc + 1) * P])
            wr[oc] = [wr1, wr2]

        # ---- x prefetch machinery ----
        xps = [None] * NIMG

        xrs = [[None, None] for _ in range(NIMG)]

        def load_dma(img, engines):
            xp = xpad_pool.tile([P, 2, PLANE_STRIDE], dt.float8e4, tag="xp", name=f"xp_{img}")
            xps[img] = xp
            for ic in range(2):
                # zero only the padding borders: [guard+top row], [bottom row
                # +tail guard], and the adjacent (right,left) pad pairs
                # between consecutive interior rows.
                nc.gpsimd.memset(xp[:, ic, 0:59], 0.0)
                nc.gpsimd.memset(xp[:, ic, 3306:PLANE_STRIDE], 0.0)
                pairs = xp[:, ic, 58 : 58 + 56 * WP].rearrange(
                    "p (r w) -> p r w", w=WP
                )[:, :, 0:2]
                nc.gpsimd.memset(pairs, 0.0)
                xr = xraw_pool.tile([P, H, W], dt.float32, tag="xr", name=f"xr_{img}_{ic}")
                engines[ic].dma_start(out=xr, in_=x[img, ic * P : (ic + 1) * P])
                xrs[img][ic] = xr

        def sign_img(img):
            xp = xps[img]
            for ic in range(2):
                interior = xp[:, ic, 1 : 1 + PLANE].rearrange(
                    "p (h w) -> p h w", w=WP
                )[:, 1 : 1 + H, 1 : 1 + W]
                nc.scalar.activation(out=interior, in_=xrs[img][ic], func=AF.Sign)

        def sign_weights(oc):
            wr1, wr2 = wr[oc]
            ws1 = wprep.tile(
                [P, 256, 3, 3], dt.bfloat16, tag="wsign", name=f"ws1_{oc}"
            )
            ws2 = wprep.tile(
                [P, 256, 3, 3], dt.bfloat16, tag="wsign2", name=f"ws2_{oc}"
            )
            nc.scalar.activation(out=ws1, in_=wr1, func=AF.Sign)
            nc.scalar.activation(out=ws2, in_=wr2, func=AF.Sign)
            wsum = wprep.tile([P, 256, 3, 3], dt.bfloat16, tag="wsum", name=f"wsum_{oc}")
            nc.vector.tensor_add(out=wsum, in0=ws1, in1=ws2)
            return wsum

        def transpose_weights(oc, wsum):
            for tap in range(9):
                ky, kx = tap // 3, tap % 3
                for ic in range(2):
                    pt = psum_pool.tile(
                        [P, P], dt.bfloat16, tag="acc", name=f"wt_{oc}_{tap}_{ic}"
                    )
                    nc.tensor.transpose(
                        pt, wsum[:, ic * P : (ic + 1) * P, ky, kx], ident
                    )
                    nc.vector.tensor_copy(out=lhsT_t[oc][tap][:, ic, :], in_=pt)

        def alpha_reduce(oc):
            wr1, wr2 = wr[oc]
            asum1 = wprep.tile([P, 1], dt.float32, tag="asum", name=f"as1_{oc}")
            asum2 = wprep.tile([P, 1], dt.float32, tag="asum2", name=f"as2_{oc}")
            for asum, w_ in ((asum1, wr1), (asum2, wr2)):
                nc.vector.tensor_reduce(
                    out=asum,
                    in_=w_[:].rearrange("p a b c -> p (a b c)"),
                    axis=mybir.AxisListType.X,
                    op=mybir.AluOpType.add,
                    apply_absolute_value=True,
                )
            nc.vector.tensor_add(out=alpha_sb[:, oc : oc + 1], in0=asum1, in1=asum2)
            nc.vector.tensor_scalar_mul(
                alpha_sb[:, oc : oc + 1], alpha_sb[:, oc : oc + 1], 1.0 / (2 * 2304)
            )

        def conv_oc(img, oc):
            xp = xps[img]
            psums = []
            for blk in range(NBLK):
                psums.append(
                    psum_pool.tile(
                        [P, N_FREE], dt.float32, tag="acc", name=f"acc_{img}_{oc}_{blk}"
                    )
                )
            for tap in range(9):
                ky, kx = tap // 3, tap % 3
                lhsT = lhsT_t[oc][tap]
                for blk in range(NBLK):
                    win = (blk * RB + ky) * WP + kx
                    nc.tensor.matmul(
                        out=psums[blk],
                        lhsT=lhsT,
                        rhs=xp[:, :, win : win + N_FREE],
                        start=(tap == 0),
                        stop=(tap == 8),
                        perf_mode=mybir.MatmulPerfMode.DoubleRow,
                    )
            for blk in range(NBLK):
                rs = blk * RB
                ot = outp.tile([P, RB, W], dt.float32, tag="ot", name=f"ot_{img}_{oc}_{blk}")
                psv = psums[blk][:].rearrange("p (h w) -> p h w", w=WP)[:, :, 1 : 1 + W]
                if blk % 2 == 0:
                    nc.vector.tensor_scalar_mul(ot, psv, alpha_sb[:, oc : oc + 1])
                else:
                    nc.scalar.activation(
                        out=ot, in_=psv, func=AF.Copy, scale=alpha_sb[:, oc : oc + 1]
                    )
                nc.sync.dma_start(
                    out=out[img, oc * P : (oc + 1) * P, rs : rs + RB, :], in_=ot
                )

        # ---- schedule ----
        # startup: weight DMAs already queued first on sync/scalar;
        # img0 x rides the gpsimd SWDGE queue, img1 behind the weights.
        load_dma(0, (nc.gpsimd, nc.gpsimd))
        load_dma(1, (nc.gpsimd, nc.gpsimd))
        wsum0 = sign_weights(0)
        wsum1 = sign_weights(1)
        transpose_weights(0, wsum0)
        transpose_weights(1, wsum1)
        sign_img(0)
        sign_img(1)
        alpha_reduce(0)
        alpha_reduce(1)
        conv_oc(0, 0)
        load_dma(2, (nc.sync, nc.scalar))
        sign_img(2)
        conv_oc(0, 1)
        conv_oc(1, 0)
        load_dma(3, (nc.sync, nc.scalar))
        sign_img(3)
        conv_oc(1, 1)
        conv_oc(2, 0)
        conv_oc(2, 1)
        conv_oc(3, 0)
        conv_oc(3, 1)

    nc.compile()
    return nc


def _get_program():
    if "nc" not in _CACHE:
        _CACHE["nc"] = _build_program()
    return _CACHE["nc"]


def _run(x, weight1, weight2, **spmd_kwargs):
    from concourse.bass_utils import run_bass_kernel_spmd

    nc = _get_program()
    x = np.ascontiguousarray(x, dtype=np.float32)
    w1 = np.ascontiguousarray(weight1, dtype=np.float32)
    w2 = np.ascontiguousarray(weight2, dtype=np.float32)
    in_maps = [
        {"x": x[i * NIMG : (i + 1) * NIMG], "w1": w1, "w2": w2} for i in range(NCORES)
    ]
    res = run_bass_kernel_spmd(nc, in_maps, list(range(NCORES)), **spmd_kwargs)
    out = np.concatenate([res.results[i]["out"] for i in range(NCORES)], axis=0)
    return out, res


def kernel(x, weight1, weight2):
    out, _ = _run(x, weight1, weight2)
    return out



# revision 10
# speedup vs baseline: 1.0095x; 1.0095x over previous
"""XNOR-Net++ style binary double-conv forward for Trainium2, 8 NeuronCores.

Computes, for x:[32,256,56,56] f32, w1,w2:[256,256,3,3] f32:
    alpha = (mean|w1| + mean|w2|)/2 over (I,kh,kw)            -> [O]
    out   = (conv(sign(x), sign(w1)) + conv(sign(x), sign(w2))) * alpha

Key algebraic fold: conv(s, sign(w1)) + conv(s, sign(w2)) ==
conv(s, sign(w1)+sign(w2)); the combined weight is in {-2,0,2} and the
activations in {-1,0,1}, both exact in fp8e4, so the conv's integer
part is reproduced exactly and only the final alpha multiply rounds.

Implementation: 9 shifted-tap matmuls accumulate each output tile in
PSUM. fp8 + DoubleRow folds the K=256 contraction (2x128 C_in chunks)
into single matmuls. The padded sign image is stored flat ([58*58] per
C_in chunk, 58-wide rows, one guard byte on each end) so every tap
window is a contiguous 464-element slice; the wrap-around columns land
in the two padding columns of each 8-row output block and are dropped
by the PSUM->SBUF scale-copy.

v2 scheduling (the conv matmul stream was already back-to-back at 202
ns/MM in the v1 trace; all headroom was the 36us pre-PE startup and
9.5us tail):
  - All input DMAs ride ONE ring (sync) in priority order: w-oc0
    quarters, img0 row-slabs (with w-oc1 quarters woven between), then
    img1..3.  Output DMAs ride the gpsimd ring.
  - Weight prep fans out across engines: w1 signs via ScalarE ACT,
    w2 signs via DVE one-pass bitwise (x AND -0.0) OR 1.0, alpha abs-
    reduces on gpsimd, sums on DVE, transposes on PE.
  - img0 is DMAd and signed in 7 row-slabs so conv blocks start before
    the image finishes loading; conv groups are block-major (psum bank
    per 8-row block, 9 tap MMs, scale-copy on DVE, DMA out).
  - ~40 dummy 128-col matmuls warm the PE HAM clock gate during the
    initial DMA window so real matmuls start at 2.4 GHz.

Sharding: data-parallel over batch, 4 images per core, weights
replicated; outputs concatenated on host.
"""

import numpy as np

P = 128
H = W = 56
WP = 58  # padded row width
PLANE = WP * WP  # 3364 flat padded plane
PLANE_STRIDE = 3376  # 16-aligned (DoubleRow AP step%16==0), >= 1+PLANE+1
NIMG = 4  # images per core
NCORES = 8
RB = 8  # output rows per matmul block
NBLK = H // RB  # 7
N_FREE = RB * WP  # 464 <= 512 (one PSUM bank)

# img0 row slabs: block b's window needs image rows 8b-1 .. 8b+8, so
# slabs [0,10),[10,18),...,[42,50),[50,56) make block b depend on
# slabs 0..b only.
SLABS = [(0, 10), (10, 18), (18, 26), (26, 34), (34, 42), (42, 50), (50, 56)]

_CACHE = {}


def _build_program():
    from contextlib import ExitStack

    import concourse.bacc as bacc
    import concourse.mybir as mybir
    import concourse.tile as tile
    from concourse.masks import make_identity

    dt = mybir.dt
    AF = mybir.ActivationFunctionType
    ALU = mybir.AluOpType

    nc = bacc.Bacc(
        "TRN2",
        target_bir_lowering=False,
        debug=False,
        num_devices=NCORES,
    )
    x = nc.dram_tensor("x", [NIMG, 256, H, W], dt.float32, kind="ExternalInput").ap()
    w1 = nc.dram_tensor("w1", [256, 256, 3, 3], dt.float32, kind="ExternalInput").ap()
    w2 = nc.dram_tensor("w2", [256, 256, 3, 3], dt.float32, kind="ExternalInput").ap()
    out = nc.dram_tensor(
        "out", [NIMG, 256, H, W], dt.float32, kind="ExternalOutput"
    ).ap()

    with tile.TileContext(nc) as tc, ExitStack() as ctx:
        consts = ctx.enter_context(tc.tile_pool(name="consts", bufs=1))
        wraw_pool = ctx.enter_context(tc.tile_pool(name="wraw", bufs=8))
        wsign_pool = ctx.enter_context(tc.tile_pool(name="wsign", bufs=4))
        wsum_pool = ctx.enter_context(tc.tile_pool(name="wsum", bufs=4))
        xraw_pool = ctx.enter_context(tc.tile_pool(name="xraw", bufs=4))
        xpad_pool = ctx.enter_context(tc.tile_pool(name="xpad", bufs=3))
        psum_pool = ctx.enter_context(tc.tile_pool(name="psum", bufs=4, space="PSUM"))
        wtps_pool = ctx.enter_context(tc.tile_pool(name="wtps", bufs=2, space="PSUM"))
        outp = ctx.enter_context(tc.tile_pool(name="outp", bufs=6))

        ident = consts.tile([P, P], dt.bfloat16)
        make_identity(nc, ident)

        # alpha per output channel, one column per oc chunk
        alpha_sb = consts.tile([P, 2], dt.float32)
        apart = consts.tile([P, 2, 4], dt.float32)  # abs-sum partials per oc
        # per-(oc,tap) stationary weight tiles [i_local, ic, o_local], fp8
        lhsT_t = [
            [
                consts.tile([P, 2, P], dt.float8e4, name=f"lhsT_{oc}_{tap}")
                for tap in range(9)
            ]
            for oc in range(2)
        ]

        # ---------------- input DMA ring (sync engine, priority order) ----
        # weight quarter-pieces [128o, 128i*9] and per-image raw tiles.
        wr = {}  # (w, oc, ic) -> [128, 1152] f32 tile

        def wpiece_dma(widx, oc, ic):
            src = (w1, w2)[widx]
            t = wraw_pool.tile(
                [P, P, 3, 3], dt.float32, tag="wraw", name=f"wr_{widx}_{oc}_{ic}"
            )
            nc.sync.dma_start(
                out=t, in_=src[oc * P : (oc + 1) * P, ic * P : (ic + 1) * P]
            )
            wr[(widx, oc, ic)] = t

        xrs = [[None, None] for _ in range(NIMG)]

        def img_raw_tile(img):
            for ic in range(2):
                xrs[img][ic] = xraw_pool.tile(
                    [P, H, W], dt.float32, tag="xr", name=f"xr_{img}_{ic}"
                )

        def slab_dma(img, s):
            r0, r1 = SLABS[s]
            for ic in range(2):
                nc.sync.dma_start(
                    out=xrs[img][ic][:, r0:r1, :],
                    in_=x[img, ic * P : (ic + 1) * P, r0:r1, :],
                )

        def img_dma(img):
            for ic in range(2):
                nc.sync.dma_start(
                    out=xrs[img][ic], in_=x[img, ic * P : (ic + 1) * P]
                )

        # ring order: w-oc0 quarters | s0,s1 | w-oc1 quarters woven with
        # s2..s5 | s6 | img1 | img2 | img3
        wpiece_dma(0, 0, 0)
        wpiece_dma(0, 0, 1)
        wpiece_dma(1, 0, 0)
        wpiece_dma(1, 0, 1)
        img_raw_tile(0)
        slab_dma(0, 0)
        slab_dma(0, 1)
        wpiece_dma(0, 1, 0)
        slab_dma(0, 2)
        wpiece_dma(0, 1, 1)
        slab_dma(0, 3)
        wpiece_dma(1, 1, 0)
        slab_dma(0, 4)
        wpiece_dma(1, 1, 1)
        slab_dma(0, 5)
        slab_dma(0, 6)
        img_raw_tile(1)
        img_dma(1)
        img_raw_tile(2)
        img_dma(2)
        img_raw_tile(3)
        img_dma(3)

        # ---------------- image pad tiles ---------------------------------
        xps = [None] * NIMG

        def img_pad_tile(img, memset_eng):
            xp = xpad_pool.tile(
                [P, 2, PLANE_STRIDE], dt.float8e4, tag="xp", name=f"xp_{img}"
            )
            xps[img] = xp
            for ic in range(2):
                memset_eng.memset(xp[:, ic, 0:59], 0.0)
                memset_eng.memset(xp[:, ic, 3306:PLANE_STRIDE], 0.0)
                pairs = xp[:, ic, 58 : 58 + 56 * WP].rearrange(
                    "p (r w) -> p r w", w=WP
                )[:, :, 0:2]
                memset_eng.memset(pairs, 0.0)

        # img0 pad borders early on idle DVE (no input deps)
        img_pad_tile(0, nc.vector)

        # ---------------- PE HAM warm-up: dummy matmuls during DMA window --
        for d in range(40):
            dps = wtps_pool.tile([P, P], dt.float32, tag="wt", name=f"dummy_{d}")
            nc.tensor.matmul(out=dps, lhsT=ident, rhs=ident, start=True, stop=True)

        # ---------------- weight prep ------------------------------------
        # w1 signs on ScalarE (ACT Sign), w2 signs on DVE via
        # (x AND -0.0) OR 1.0 == copysign(1.0, x); adds on DVE -> bf16 wsum;
        # alpha abs-reduces on gpsimd; transposes on PE; psum->lhsT copies
        # split gpsimd/DVE.
        wsums = {}  # (oc, ic) -> [128o, 128i, 3, 3] bf16

        def prep_piece_signs(oc, ic):
            s1 = wsign_pool.tile(
                [P, P, 3, 3], dt.float32, tag="ws1", name=f"ws1_{oc}_{ic}"
            )
            nc.scalar.activation(out=s1, in_=wr[(0, oc, ic)], func=AF.Sign)
            s2 = wsign_pool.tile(
                [P, P, 3, 3], dt.float32, tag="ws2", name=f"ws2_{oc}_{ic}"
            )
            # copysign(1.0, w2) in one DVE pass: (bits & 0x80000000) | 0x3F800000
            nc.vector.tensor_scalar(
                out=s2[:].bitcast(dt.int32),
                in0=wr[(1, oc, ic)][:].bitcast(dt.int32),
                scalar1=-2147483648,
                scalar2=1065353216,
                op0=ALU.bitwise_and,
                op1=ALU.bitwise_or,
            )
            ws = wsum_pool.tile(
                [P, P, 3, 3], dt.bfloat16, tag="wsum", name=f"wsum_{oc}_{ic}"
            )
            nc.vector.tensor_add(out=ws, in0=s1, in1=s2)
            wsums[(oc, ic)] = ws

        def alpha_reduce_piece(widx, oc, ic):
            nc.vector.tensor_reduce(
                out=apart[:, oc, 2 * widx + ic : 2 * widx + ic + 1],
                in_=wr[(widx, oc, ic)][:].rearrange("p a b c -> p (a b c)"),
                axis=mybir.AxisListType.X,
                op=ALU.add,
                apply_absolute_value=True,
            )

        def alpha_combine(oc):
            nc.vector.tensor_reduce(
                out=alpha_sb[:, oc : oc + 1],
                in_=apart[:, oc],
                axis=mybir.AxisListType.X,
                op=ALU.add,
            )
            nc.vector.tensor_scalar_mul(
                alpha_sb[:, oc : oc + 1], alpha_sb[:, oc : oc + 1], 1.0 / (2 * 2304)
            )

        def transpose_ic(oc, ic):
            ws = wsums[(oc, ic)]
            for tap in range(9):
                ky, kx = tap // 3, tap % 3
                pt = wtps_pool.tile(
                    [P, P], dt.bfloat16, tag="wt", name=f"wt_{oc}_{tap}_{ic}"
                )
                nc.tensor.transpose(pt, ws[:, :, ky, kx], ident)
                nc.vector.tensor_copy(out=lhsT_t[oc][tap][:, ic, :], in_=pt)

        # oc0 prep (pieces arrive ~7-12us); alpha reduces issued after the
        # sign->add->transpose chain (first scale-copy tolerates late alpha)
        prep_piece_signs(0, 0)
        transpose_ic(0, 0)
        prep_piece_signs(0, 1)
        transpose_ic(0, 1)
        alpha_reduce_piece(0, 0, 0)
        alpha_reduce_piece(0, 0, 1)
        alpha_reduce_piece(1, 0, 0)
        alpha_reduce_piece(1, 0, 1)
        alpha_combine(0)

        # ---------------- image sign pipeline ----------------------------
        def sign_slab(img, s):
            r0, r1 = SLABS[s]
            xp = xps[img]
            for ic in range(2):
                interior = xp[:, ic, 1 : 1 + PLANE].rearrange(
                    "p (h w) -> p h w", w=WP
                )[:, 1 + r0 : 1 + r1, 1 : 1 + W]
                nc.scalar.activation(
                    out=interior, in_=xrs[img][ic][:, r0:r1, :], func=AF.Sign
                )

        def sign_img(img):
            xp = xps[img]
            for ic in range(2):
                interior = xp[:, ic, 1 : 1 + PLANE].rearrange(
                    "p (h w) -> p h w", w=WP
                )[:, 1 : 1 + H, 1 : 1 + W]
                nc.scalar.activation(out=interior, in_=xrs[img][ic], func=AF.Sign)

        for s in range(7):
            sign_slab(0, s)

        # ---------------- conv groups (block-major) ----------------------
        def conv_oc(img, oc):
            xp = xps[img]
            for blk in range(NBLK):
                ps = psum_pool.tile(
                    [P, N_FREE], dt.float32, tag="acc", name=f"acc_{img}_{oc}_{blk}"
                )
                for tap in range(9):
                    ky, kx = tap // 3, tap % 3
                    win = (blk * RB + ky) * WP + kx
                    nc.tensor.matmul(
                        out=ps,
                        lhsT=lhsT_t[oc][tap],
                        rhs=xp[:, :, win : win + N_FREE],
                        start=(tap == 0),
                        stop=(tap == 8),
                        perf_mode=mybir.MatmulPerfMode.DoubleRow,
                    )
                rs = blk * RB
                ot = outp.tile(
                    [P, RB, W], dt.float32, tag="ot", name=f"ot_{img}_{oc}_{blk}"
                )
                psv = ps[:].rearrange("p (h w) -> p h w", w=WP)[:, :, 1 : 1 + W]
                nc.vector.tensor_scalar_mul(ot, psv, alpha_sb[:, oc : oc + 1])
                nc.gpsimd.dma_start(
                    out=out[img, oc * P : (oc + 1) * P, rs : rs + RB, :], in_=ot
                )

        # group (0,0) runs while img0 slabs stream in
        conv_oc(0, 0)

        # oc1 weight prep (pieces arrive ~16-25us, overlaps group (0,0))
        prep_piece_signs(1, 0)
        alpha_reduce_piece(0, 1, 0)
        alpha_reduce_piece(0, 1, 1)
        transpose_ic(1, 0)
        prep_piece_signs(1, 1)
        alpha_reduce_piece(1, 1, 0)
        alpha_reduce_piece(1, 1, 1)
        alpha_combine(1)
        transpose_ic(1, 1)

        img_pad_tile(1, nc.vector)
        sign_img(1)
        conv_oc(0, 1)
        img_pad_tile(2, nc.vector)
        sign_img(2)
        conv_oc(1, 0)
        conv_oc(1, 1)
        img_pad_tile(3, nc.vector)
        sign_img(3)
        conv_oc(2, 0)
        conv_oc(2, 1)
        conv_oc(3, 0)
        conv_oc(3, 1)

    nc.compile()
    return nc


def _get_program():
    if "nc" not in _CACHE:
        _CACHE["nc"] = _build_program()
    return _CACHE["nc"]


def _run(x, weight1, weight2, **spmd_kwargs):
    from concourse.bass_utils import run_bass_kernel_spmd

    nc = _get_program()
    x = np.ascontiguousarray(x, dtype=np.float32)
    w1 = np.ascontiguousarray(weight1, dtype=np.float32)
    w2 = np.ascontiguousarray(weight2, dtype=np.float32)
    in_maps = [
        {"x": x[i * NIMG : (i + 1) * NIMG], "w1": w1, "w2": w2} for i in range(NCORES)
    ]
    res = run_bass_kernel_spmd(nc, in_maps, list(range(NCORES)), **spmd_kwargs)
    out = np.concatenate([res.results[i]["out"] for i in range(NCORES)], axis=0)
    return out, res


def kernel(x, weight1, weight2):
    out, _ = _run(x, weight1, weight2)
    return out
